# revision 1
# baseline (speedup 1.0000x reference)
"""Causal self-attention (B=2, L=2048, C=2048, H=16) on 8 trn2 NeuronCores.

Sharding: tensor-parallel over heads — 2 heads per core. Each core computes
its heads' q/k/v from the full x, runs causal attention, and produces a
partial y @ w_proj[:, its-cols].T; the host sums the 8 partials.

Notes:
- The reference's RoPE rotates q and k by identical per-head (position-
  independent) angles; an orthogonal rotation applied to both sides leaves
  q.k unchanged, so RoPE is skipped entirely.
- Matmuls run in fp32r (full-rate fp32 on the PE when the moving dim >= 256).
  P (softmax weights) and V use bf16 for the attention-value matmul.
- Softmax skips the max-subtraction (scores are ~N(0,1); exp is safe) and
  folds the denominator into a ones-vector matmul accumulated on the PE.
"""
import sys
sys.path.insert(0, '/opt/trn_rl_repo')
import contextlib
import ctypes
import os
import types

import numpy as np
import ml_dtypes

import concourse.bacc as bacc
import concourse.tile as tile
from concourse import mybir
from concourse.bass_utils import run_bass_kernel_spmd

F32 = mybir.dt.float32
F32R = mybir.dt.float32r
BF16 = mybir.dt.bfloat16
AF = mybir.ActivationFunctionType

B, L, C, H, D = 2, 2048, 2048, 16, 128
NCORES = 8
HPC = H // NCORES            # heads per core
TC = 256                     # phase-A token chunk
NCH = L // TC                # chunks per batch
KT16 = C // 128              # contraction tiles over C
SCALE = 1.0 / float(np.sqrt(D))

LAST_RESULT = None           # BassKernelResults of the most recent run


def _install_ntff_shim():
    """Register the axon NTFF profile hook so BASS_TRACE=1 yields exec_time_ns."""
    if "antenv.axon_hooks" in sys.modules:
        return
    so_path = "/opt/axon/libaxon_pjrt.so"
    if not os.path.exists(so_path):
        return
    lib = ctypes.CDLL(so_path)
    if not hasattr(lib, "axon_start_nrt_profile"):
        return
    lib.axon_start_nrt_profile.argtypes = [ctypes.POINTER(ctypes.c_int64), ctypes.c_size_t]
    lib.axon_start_nrt_profile.restype = ctypes.c_int64
    lib.axon_stop_nrt_profile.argtypes = [ctypes.c_char_p]
    lib.axon_stop_nrt_profile.restype = ctypes.c_int64

    @contextlib.contextmanager
    def _hook(output_dir, device_ids):
        import jax
        jax.devices()
        if device_ids:
            ids = (ctypes.c_int64 * len(device_ids))(*device_ids)
            rc = lib.axon_start_nrt_profile(ids, len(device_ids))
        else:
            rc = lib.axon_start_nrt_profile(None, 0)
        if rc != 0:
            raise RuntimeError(f"axon_start_nrt_profile rc={rc}")
        try:
            yield
        finally:
            n = lib.axon_stop_nrt_profile(str(output_dir).encode())
            if n <= 0:
                print(f"ntff capture wrote {n} files to {output_dir}")

    mod = types.ModuleType("antenv.axon_hooks")
    mod.get_axon_ntff_profile_hook = lambda: _hook
    mod.set_axon_ntff_profile_hook = lambda h: None
    sys.modules["antenv.axon_hooks"] = mod


def _build():
    nc = bacc.Bacc()
    xT = nc.dram_tensor("xT", [C, B * L], F32, kind="ExternalInput")
    wqkT = nc.dram_tensor("wqkT", [C, 4 * D], F32, kind="ExternalInput")
    wvT = nc.dram_tensor("wvT", [C, 2 * D], F32, kind="ExternalInput")
    wpT = nc.dram_tensor("wpT", [2 * D, C], F32, kind="ExternalInput")
    maskd = nc.dram_tensor("maskd", [128, 4, 512], BF16, kind="ExternalInput")
    onesd = nc.dram_tensor("onesd", [128, 1], BF16, kind="ExternalInput")
    outd = nc.dram_tensor("out", [B * L, C], F32, kind="ExternalOutput")

    xR = xT.rearrange("(t p) n -> t p n", p=128)      # [16, 128, B*L]
    wqkR = wqkT.rearrange("(t p) n -> t p n", p=128)  # [16, 128, 512]
    wvR = wvT.rearrange("(t p) n -> t p n", p=128)    # [16, 128, 256]
    wpR = wpT.rearrange("(t p) n -> t p n", p=128)    # [2, 128, 2048]

    with tile.TileContext(nc) as tc:
        with tc.tile_pool(name="consts", bufs=1) as cp, \
             tc.tile_pool(name="big", bufs=1) as bp, \
             tc.tile_pool(name="xp", bufs=2) as xp, \
             tc.tile_pool(name="ptp", bufs=6) as ptp, \
             tc.tile_pool(name="smp", bufs=2) as smp, \
             tc.tile_pool(name="osp", bufs=3) as osp, \
             tc.tile_pool(name="ps", bufs=4, space="PSUM") as ps, \
             tc.tile_pool(name="psa", bufs=2, space="PSUM") as psa:

            # PE warm-up: ~3.5us of matmuls on memset data so the HAM clock
            # gate reaches 8/8 before the first real matmuls arrive.
            warm = cp.tile([128, 256], BF16)
            nc.vector.memset(warm, 0.0)
            pw = ps.tile([128, 256], F32, tag="mm")
            NWARM = 140   # ~15us: keeps the PE busy while weights stream in
            for i in range(NWARM):
                nc.tensor.matmul(pw, warm[:, :128], warm,
                                 start=(i == 0), stop=(i == NWARM - 1))

            # Weights/constants go on the ACT hwdge queue so the sync queue is
            # dedicated to x-chunk streaming. wqk is split per (k, m) so the
            # m=0 accumulation chain can start before all weights have landed.
            wqk = cp.tile([128, KT16, 4 * D], F32R)
            wv = cp.tile([128, KT16, 2 * D], F32R)
            wp = cp.tile([128, 2, C], F32R)
            wqkP = wqkR.transpose([1, 0, 2])  # [128, 16, 512]
            wvP = wvR.transpose([1, 0, 2])    # [128, 16, 256]
            # order: m=0 weights, then wv (chunk-0 computes m=0 then V while
            # the remaining m=1..3 weights stream in)
            for m in [0, None, 1]:
                if m is None:
                    for k2 in range(KT16 // 2):
                        nc.scalar.dma_start(out=wv[:, 2 * k2:2 * k2 + 2],
                                            in_=wvP[:, 2 * k2:2 * k2 + 2].bitcast(F32R))
                    continue
                for k2 in range(KT16 // 2):
                    nc.scalar.dma_start(
                        out=wqk[:, 2 * k2:2 * k2 + 2, m * 128:(m + 1) * 128],
                        in_=wqkP[:, 2 * k2:2 * k2 + 2,
                                 m * 128:(m + 1) * 128].bitcast(F32R))
            for j in range(2):
                nc.scalar.dma_start(out=wp[:, j], in_=wpR[j].bitcast(F32R))
            tm = cp.tile([128, 4, 512], BF16)
            nc.scalar.dma_start(out=tm, in_=maskd[:, :, :])
            tones = cp.tile([128, 1], BF16)
            nc.scalar.dma_start(out=tones, in_=onesd[:, :])

            for b in range(B):
                # ---- Phase A: q/k/v projections for this batch ----
                QT = bp.tile([128, HPC, L], F32R, tag="QT")   # [d, hi, tok]
                KT = bp.tile([128, HPC, L], F32R, tag="KT")
                V = bp.tile([128, L // 128, 2 * D], BF16, tag="V")  # [tok, tt, hi*D]
                for ch in range(NCH):
                    t0 = b * L + ch * TC
                    xc = xp.tile([128, KT16, TC], F32R)
                    for k2 in range(KT16 // 2):
                        nc.sync.dma_start(
                            out=xc[:, 2 * k2:2 * k2 + 2],
                            in_=xR.transpose([1, 0, 2])[:, 2 * k2:2 * k2 + 2,
                                                        t0:t0 + TC].bitcast(F32R))
                    if b == 0 and ch == 0:
                        # late weights ride the sync queue behind chunk 0
                        for mw in (2, 3):
                            for k2 in range(KT16 // 2):
                                nc.sync.dma_start(
                                    out=wqk[:, 2 * k2:2 * k2 + 2, mw * 128:(mw + 1) * 128],
                                    in_=wqkP[:, 2 * k2:2 * k2 + 2,
                                             mw * 128:(mw + 1) * 128].bitcast(F32R))
                    order = [0, 'V', 1, 2, 3] if (b == 0 and ch == 0) else [0, 1, 2, 3, 'V']
                    for m in order:
                        if m == 'V':
                            for tt in range(TC // 128):
                                pv = ps.tile([128, 2 * D], F32, tag="mm")
                                for k in range(KT16):
                                    nc.tensor.matmul(pv, xc[:, k, tt * 128:(tt + 1) * 128],
                                                     wv[:, k], start=(k == 0),
                                                     stop=(k == KT16 - 1))
                                nc.vector.tensor_copy(V[:, ch * (TC // 128) + tt], pv)
                            continue
                        pq = ps.tile([128, TC], F32, tag="mm")
                        for k in range(KT16):
                            nc.tensor.matmul(pq, wqk[:, k, m * 128:(m + 1) * 128],
                                             xc[:, k], start=(k == 0), stop=(k == KT16 - 1))
                        dst = QT if m < 2 else KT
                        nc.vector.tensor_copy(dst[:, m % 2, ch * TC:(ch + 1) * TC], pq)

                # ---- Phase B: causal attention per head ----
                yT = bp.tile([128, HPC, L], F32R, tag="yT")   # [d, hi, tok]
                # Phase B software pipeline, two heads interleaved so the PE
                # always has an independent chain to run while ACT computes
                # exp for the other head. S^T matmuls run 2 items ahead.
                # Phase C matmul pairs are injected into the stream as PE
                # filler one q-group after their yT inputs were normalized.
                # Diagonal k-tiles only touch their valid columns [off, 512).
                items = [(hi, g, kt)
                         for g in range(4)
                         for kt in range(4 * (g + 1))
                         for hi in range(HPC)]

                def s_matmul(hi, g, kt):
                    off = max(0, 128 * (kt - 4 * g))
                    pss = ps.tile([128, 512], F32, tag="mm")
                    nc.tensor.matmul(pss[:, off:], KT[:, hi, kt * 128:(kt + 1) * 128],
                                     QT[:, hi, g * 512 + off:(g + 1) * 512],
                                     start=True, stop=True)
                    return pss

                def proj_pair(tt, nch):
                    po = ps.tile([128, 512], F32, tag="mm")
                    for hi in range(HPC):
                        nc.tensor.matmul(po, yT[:, hi, tt * 128:(tt + 1) * 128],
                                         wp[:, hi, nch * 512:(nch + 1) * 512],
                                         start=(hi == 0), stop=(hi == HPC - 1))
                    ot = osp.tile([128, 512], F32)
                    nc.vector.tensor_copy(ot, po)
                    nc.gpsimd.dma_start(
                        out=outd[b * L + tt * 128: b * L + (tt + 1) * 128,
                                 nch * 512:(nch + 1) * 512],
                        in_=ot)

                pss_q = [s_matmul(*items[0]), s_matmul(*items[1])]
                psy = {}
                psr = {}
                done_heads = {}
                pending_c = []
                for i, (hi, g, kt) in enumerate(items):
                    nkt = 4 * (g + 1)
                    off = max(0, 128 * (kt - 4 * g))
                    if kt == 0:
                        psy[hi] = psa.tile([128, 512], F32, tag="acc", name=f"psy{hi}")
                        psr[hi] = psa.tile([1, 512], F32, tag="rs", name=f"psr{hi}")
                    pss = pss_q.pop(0)
                    ptile = ptp.tile([128, 512], BF16)
                    nc.scalar.activation(ptile[:, off:], pss[:, off:], AF.Exp, scale=SCALE)
                    if i + 2 < len(items):
                        pss_q.append(s_matmul(*items[i + 2]))
                    if kt >= 4 * g:
                        nc.vector.tensor_mul(ptile[:, off:], ptile[:, off:],
                                             tm[:, kt - 4 * g, off:])
                    nc.tensor.matmul(psy[hi][:, off:], V[:, kt, hi * D:(hi + 1) * D],
                                     ptile[:, off:],
                                     start=(kt == 0), stop=(kt == nkt - 1),
                                     skip_group_check=True)
                    nc.tensor.matmul(psr[hi][:, off:], tones, ptile[:, off:],
                                     start=(kt == 0), stop=(kt == nkt - 1),
                                     skip_group_check=True)
                    if kt == nkt - 1:
                        rs = smp.tile([1, 512], F32, tag="rs_sb")
                        nc.vector.reciprocal_approx_fast(out=rs, in_=psr[hi])
                        rb = smp.tile([128, 512], F32, tag="rb")
                        nc.gpsimd.partition_broadcast(rb, rs)
                        nc.vector.tensor_mul(yT[:, hi, g * 512:(g + 1) * 512],
                                             psy[hi], rb)
                        done_heads[g] = done_heads.get(g, 0) + 1
                        if done_heads[g] == HPC:
                            pending_c.extend((tt, nch)
                                             for tt in range(4 * g, 4 * g + 4)
                                             for nch in range(4))
                    elif pending_c:
                        proj_pair(*pending_c.pop(0))
                for tt, nch in pending_c:
                    proj_pair(tt, nch)
    nc.compile()
    return nc


def _make_masks():
    masks = np.zeros((128, 4, 512), dtype=np.float32)
    kk = np.arange(128)[:, None]
    qq = np.arange(128)[None, :]
    tri = (kk <= qq).astype(np.float32)
    for p in range(4):
        for j in range(4):
            blk = masks[:, p, j * 128:(j + 1) * 128]
            if j > p:
                blk[:] = 1.0
            elif j == p:
                blk[:] = tri
    return masks.astype(ml_dtypes.bfloat16)


_cached_nc = None


def kernel(x, w_attn, w_proj):
    global _cached_nc, LAST_RESULT
    if os.environ.get("BASS_TRACE"):
        _install_ntff_shim()
    if _cached_nc is None:
        _cached_nc = _build()
    nc = _cached_nc

    x = np.asarray(x, dtype=np.float32)
    w_attn = np.asarray(w_attn, dtype=np.float32)
    w_proj = np.asarray(w_proj, dtype=np.float32)

    xT = np.ascontiguousarray(x.reshape(B * L, C).T)
    masks = _make_masks()
    ones = np.ones((128, 1), dtype=ml_dtypes.bfloat16)

    in_maps = []
    for c in range(NCORES):
        h0 = HPC * c
        wq = w_attn[h0 * D:(h0 + HPC) * D]
        wk = w_attn[C + h0 * D: C + (h0 + HPC) * D]
        wv = w_attn[2 * C + h0 * D: 2 * C + (h0 + HPC) * D]
        in_maps.append({
            "xT": xT,
            "wqkT": np.ascontiguousarray(np.concatenate([wq, wk], axis=0).T),
            "wvT": np.ascontiguousarray(wv.T),
            "wpT": np.ascontiguousarray(w_proj[:, h0 * D:(h0 + HPC) * D].T),
            "maskd": masks,
            "onesd": ones,
        })

    res = run_bass_kernel_spmd(nc, in_maps, core_ids=list(range(NCORES)))
    LAST_RESULT = res
    acc = res.results[0]["out"].copy()
    for i in range(1, NCORES):
        acc += res.results[i]["out"]
    return acc.reshape(B, L, C)

